# revision 14
# baseline (speedup 1.0000x reference)
"""Fused CE + supervised-contrastive loss on 8 Trainium2 NeuronCores.

Math (reference semantics):
  ce   = -mean_i log_softmax(input)[i, y_i]
  sim  = (X @ X.T) / tau, diag excluded
  lse_i = logsumexp_{k!=i} sim[i,k]
  possum_i = (x_i . S_{y_i} - ||x_i||^2)/tau,  S_c = sum_{y_k=c} x_k
  per_i = lse_i - possum_i/n_pos_i (0 if n_pos_i == 0)
  loss = (1-lmbd)*ce + lmbd * sum_i per_i

Strategy (v2, symmetric-pair + tile-parallel PE):
  The only O(N^2) work is the exp row-sums se_i = sum_k exp(sim_ik - SHIFT).
  Each core owns 8 contiguous 128-row blocks. Row-block I processes a
  wrap-around window of W=40 column blocks j=0..39:
    - j=0: diagonal block, self-sim killed in PSUM by two 64-contraction
      "eye" accumulate-matmuls (quadrant trick keeps 64x128 tile mode).
    - j=1..24 ("symmetric"): exp'd once; row-sums via ACT/DVE accumulate,
      column-sums (the mirrored pairs' row-sums) via ones-at-s packed
      matmuls: lhsT has its ones in column s, so PSUM row s accumulates the
      strip's colsum while other rows get +0 -- 48 strips pack into one
      [128,512] PSUM tile per contraction half, drained once at the end.
    - j=25..39 ("both-directions"): distances 25..32 are computed from both
      sides (j=d and j=64-d), so no colsum is needed.
  Every matmul is 64-contraction in 64x128 tile mode: T0 instructions use
  SBUF partitions 0-63, T8 partitions 64-127, and the two tiles execute
  CONCURRENTLY (measured 2x; mode switches wedge the device, so the kernel
  never leaves 64x128 mode).  X^T/sqrt(tau) is duplicated into both
  partition halves so any chunk can go to either tile.

  exp work is split ACT (exact spline exp, ~1.2ns/col incl. overhead) vs
  DVE (Schraudolph bf16 fast-exp: clamp+scale, +bias -> int16 bitcast,
  reduce; ~2.9ns/col) to run both engines in parallel.  DVE terms carry a
  calibrated ~0.2% bias -> ~1e-5 on the final loss (tolerance 2e-2).

  Host does O(N*C) prep/post: class sums S -> G=S[y] upload, and the final
  per-row assembly (ln(se), possum, CE combine) in float64.
"""

import numpy as np

N, C = 8192, 64
NCORES = 8
RPC = N // NCORES          # 1024 rows per core
P = 128
NBLK = RPC // P            # 8 row-blocks per core
TAU = 0.5
LMBD = 0.5
SHIFT = 100.0

SYMJ = 28                  # colsum distances 1..SYMJ
WINB = 64 - SYMJ           # 36 window blocks (j = 0..35)
WCOLS = WINB * P           # 4608 window cols
CS_COLS = SYMJ * P         # 3584 colsum cols -> 7 strips of 512
STRIPS = CS_COLS // 512    # 7
XTW = (NBLK - 1) * P + WCOLS  # 7*128 + 4608 = 5504

# window = 3 chunks of 1536; chunk->tile alternates with window parity so
# both tiles carry equal sim+diag work over the 8 windows
C0, C1, C2 = 1536, 1536, 1536
DVE_C2 = 1280              # DVE takes c2[0:1280], ACT the rest

# Schraudolph bf16 fast-exp constants (calibrated)
A16 = 128.0 / float(np.log(2.0))
SIGMA = 3.25
B16P = 16256.0 - SIGMA - SHIFT * A16
CLAMP_LO = SHIFT - 88.0    # 12.0

_CACHE = {}


def _build():
    from contextlib import ExitStack

    import concourse.bass as bass
    import concourse.tile as tile
    from concourse import bacc, mybir

    f32 = mybir.dt.float32
    i16 = mybir.dt.int16
    bf16 = mybir.dt.bfloat16
    AF = mybir.ActivationFunctionType
    ALU = mybir.AluOpType
    AX = mybir.AxisListType

    nc = bacc.Bacc(
        "TRN2",
        target_bir_lowering=False,
        debug=False,
        num_devices=NCORES,
    )

    xt2_d = nc.dram_tensor("xt2", [P, XTW], bf16, kind="ExternalInput")
    xrows_d = nc.dram_tensor("xrows", [P, NBLK * C], bf16, kind="ExternalInput")
    g_d = nc.dram_tensor("g", [P, NBLK * C], bf16, kind="ExternalInput")
    zwin_d = nc.dram_tensor("zwin", [P, 184], bf16, kind="ExternalInput")
    dkit_d = nc.dram_tensor("dkit", [P, 4 * P], bf16, kind="ExternalInput")
    stats_d = nc.dram_tensor("stats", [P, 48], f32, kind="ExternalOutput")
    csa_d = nc.dram_tensor("csa", [P, 512], f32, kind="ExternalOutput")
    csb_d = nc.dram_tensor("csb", [P, 512], f32, kind="ExternalOutput")

    def emit(tc, ctx):
        const = ctx.enter_context(tc.tile_pool(name="const", bufs=1))
        psum = ctx.enter_context(tc.tile_pool(name="psum", bufs=1, space="PSUM"))
        epool = ctx.enter_context(tc.tile_pool(name="epool", bufs=2))
        m1p = ctx.enter_context(tc.tile_pool(name="m1p", bufs=2))
        pmp = ctx.enter_context(tc.tile_pool(name="pmp", bufs=2))
        stats = ctx.enter_context(tc.tile_pool(name="stats", bufs=1))

        # ---- input DMAs, ordered so window 0's operands land first ----
        xt2_sb = const.tile([P, XTW], bf16)
        nc.sync.dma_start(xt2_sb[:, 0:640], xt2_d.ap()[:, 0:640])
        nc.sync.dma_start(xt2_sb[:, 640:1664], xt2_d.ap()[:, 640:1664])
        dkit_sb = const.tile([P, 4 * P], bf16)
        nc.sync.dma_start(dkit_sb[:], dkit_d.ap())
        nc.sync.dma_start(xt2_sb[:, 1664:3712], xt2_d.ap()[:, 1664:3712])
        zwin_sb = const.tile([P, 184], bf16)
        nc.sync.dma_start(zwin_sb[:], zwin_d.ap())
        nc.sync.dma_start(xt2_sb[:, 3712:XTW], xt2_d.ap()[:, 3712:XTW])
        xrows_sb = const.tile([P, NBLK * C], bf16)
        nc.sync.dma_start(xrows_sb[:], xrows_d.ap())
        g_sb = const.tile([P, NBLK * C], bf16)
        nc.sync.dma_start(g_sb[:], g_d.ap())

        # ---- persistent tiles ----
        nshift = stats.tile([P, 1], f32)
        nc.vector.memset(nshift[:], -SHIFT)
        st = stats.tile([P, 48], f32)
        # prefetch the exp spline tables during the DMA head
        warm = stats.tile([P, 1], f32)
        nc.scalar.activation(warm[:], nshift[:], AF.Exp)

        pT0 = psum.tile([P, C0], f32, name="pT0")
        pT8 = psum.tile([P, C1], f32, name="pT8")
        csA = psum.tile([P, 512], f32, name="csA")
        csB = psum.tile([P, 512], f32, name="csB")

        dk = dkit_sb

        def colsum_strips(w, E):
            # strips for window w read E regions produced ~a window ago, so
            # emitting them inside window w+1 keeps the in-order PE queue
            # from stalling on the freshest exp region
            for t in range(STRIPS):
                s = STRIPS * w + t
                a = P + 512 * t
                nc.tensor.matmul(
                    csA[:], lhsT=zwin_sb[0:64, 55 - s:55 - s + P],
                    rhs=E[0:64, a:a + 512],
                    start=(s == 0), stop=(s == STRIPS * NBLK - 1),
                    skip_group_check=True,
                )
                nc.tensor.matmul(
                    csB[:], lhsT=zwin_sb[64:128, 55 - s:55 - s + P],
                    rhs=E[64:128, a:a + 512],
                    start=(s == 0), stop=(s == STRIPS * NBLK - 1),
                    skip_group_check=True,
                )

        prevE = None
        for w in range(NBLK):
            base = w * P
            E = epool.tile([P, WCOLS], bf16, tag="E", name="E")
            Ei = E[:].bitcast(i16)
            lo, hi = (0, 64) if w % 2 == 0 else (64, 128)
            pA, pB = (pT0, pT8) if w % 2 == 0 else (pT8, pT0)
            lhsA = xt2_sb[lo:hi, base:base + P]
            lhsB = xt2_sb[128 - hi:128 - lo, base:base + P]

            # c0 (tile A): window cols [0:1536)
            for k in range(3):
                a = k * 512
                nc.tensor.matmul(
                    pA[:, a:a + 512], lhsT=lhsA,
                    rhs=xt2_sb[lo:hi, base + a:base + a + 512],
                    start=True, stop=True,
                )
            # diagonal kill on cols [0:128), same contraction half as c0
            nc.tensor.matmul(
                pA[:, 0:P], lhsT=dkit_sb[lo:hi, 0:P],
                rhs=dkit_sb[lo:hi, P:2 * P],
                start=False, stop=True, skip_group_check=True,
            )
            nc.tensor.matmul(
                pA[:, 0:P], lhsT=dkit_sb[lo:hi, 2 * P:3 * P],
                rhs=dkit_sb[lo:hi, 3 * P:4 * P],
                start=False, stop=True, skip_group_check=True,
            )
            # c1 (tile B): window cols [1536:3072)
            for k in range(3):
                a = k * 512
                nc.tensor.matmul(
                    pB[:, a:a + 512], lhsT=lhsB,
                    rhs=xt2_sb[128 - hi:128 - lo, base + C0 + a:base + C0 + a + 512],
                    start=True, stop=True,
                )
            # ACT exp c0 -> E[0:1536)
            nc.scalar.activation(
                E[:, 0:C0], pA[:], AF.Exp, bias=nshift[:],
                accum_out=st[:, w * 4:w * 4 + 1],
            )
            # ACT exp c1 (full) -> E[1536:3072)
            nc.scalar.activation(
                E[:, C0:C0 + C1], pB[:], AF.Exp,
                bias=nshift[:], accum_out=st[:, w * 4 + 1:w * 4 + 2],
            )
            # c2 (tile A): window cols [3072:4608)
            for k in range(3):
                a = k * 512
                nc.tensor.matmul(
                    pA[:, a:a + 512], lhsT=lhsA,
                    rhs=xt2_sb[lo:hi, base + C0 + C1 + a:base + C0 + C1 + a + 512],
                    start=True, stop=True,
                )
            # DVE fast-exp c2[0:1280) (its PSUM release only gates c1 of the
            # NEXT window, which sits far down the in-order PE queue)
            m1a = m1p.tile([P, DVE_C2], f32, tag="m1a", name="m1a")
            nc.vector.tensor_scalar(
                out=m1a[:], in0=pA[:, 0:DVE_C2],
                scalar1=CLAMP_LO, scalar2=A16, op0=ALU.max, op1=ALU.mult,
            )
            nc.vector.tensor_scalar(
                out=Ei[:, C0 + C1:C0 + C1 + DVE_C2], in0=m1a[:],
                scalar1=B16P, scalar2=None, op0=ALU.add,
            )
            nc.vector.reduce_sum(
                st[:, w * 4 + 2:w * 4 + 3], E[:, C0 + C1:C0 + C1 + DVE_C2],
                axis=AX.X,
            )
            # ACT exp c2 tail [1280:1536)
            nc.scalar.activation(
                E[:, C0 + C1 + DVE_C2:WCOLS], pA[:, DVE_C2:C2], AF.Exp,
                bias=nshift[:], accum_out=st[:, w * 4 + 3:w * 4 + 4],
            )
            # previous window's colsum strips fill the PE while ACT drains c2
            if prevE is not None:
                colsum_strips(w - 1, prevE)
            prevE = E
            # possum dot: x_i . G_i for this block
            pm = pmp.tile([P, C], f32, tag="pm", name="pm")
            nc.gpsimd.tensor_mul(
                pm[:], xrows_sb[:, w * C:(w + 1) * C],
                g_sb[:, w * C:(w + 1) * C],
            )
            nc.vector.reduce_sum(st[:, 32 + w:33 + w], pm[:], axis=AX.X)

            if w == 1:
                # CE denominators, emitted early so they hide under the PE
                cescr = epool.tile([P, NBLK * C], f32, tag="cescr",
                                   name="cescr")
                nc.scalar.activation(cescr[:], xrows_sb[:], AF.Exp)
                for v in range(NBLK):
                    nc.vector.reduce_sum(
                        st[:, 40 + v:41 + v], cescr[:, v * C:(v + 1) * C],
                        axis=AX.X,
                    )

        colsum_strips(NBLK - 1, prevE)

        nc.sync.dma_start(stats_d.ap(), st[:])
        csa_sb = stats.tile([P, 512], f32)
        nc.vector.tensor_copy(csa_sb[:], csA[:])
        nc.sync.dma_start(csa_d.ap(), csa_sb[:])
        csb_sb = stats.tile([P, 512], f32)
        nc.vector.tensor_copy(csb_sb[:], csB[:])
        nc.sync.dma_start(csb_d.ap(), csb_sb[:])

    with tile.TileContext(nc) as tc, ExitStack() as ctx:
        emit(tc, ctx)

    nc.compile()
    return nc


def _get_nc(**kw):
    key = repr(sorted(kw.items()))
    if key not in _CACHE:
        _CACHE[key] = _build(**kw)
    return _CACHE[key]


def _prep(X, y):
    import ml_dtypes

    bf = ml_dtypes.bfloat16
    X = np.ascontiguousarray(np.asarray(X, dtype=np.float32))
    y = np.asarray(y).astype(np.int64).ravel()
    assert X.shape == (N, C) and y.shape == (N,)

    xs = (X.T / np.float32(np.sqrt(TAU))).astype(np.float32)  # [C, N]

    # class sums and per-row gathers (host, O(N*C))
    S = np.zeros((C, C), np.float64)
    np.add.at(S, y, X.astype(np.float64))
    G = S[y].astype(np.float32)                                # [N, C]
    cnt = np.bincount(y, minlength=C)[y].astype(np.float64)    # incl self
    nrm = (X.astype(np.float64) ** 2).sum(1)
    logit = X[np.arange(N), y].astype(np.float64)

    # ones-at-s window for colsum packing: col 55 is all-ones
    zwin = np.zeros((P, 184), np.float32)
    zwin[:, 55] = 1.0

    # diag-kill quadrants (64-contraction), duplicated into both halves so
    # either tile's contraction range can apply them
    dkit = np.zeros((P, 4 * P), np.float32)
    e64 = np.eye(64, dtype=np.float32)
    dkit[0:64, 0:64] = e64               # lhsT TL: col p = e_p (p<64)
    dkit[0:64, P:P + 64] = -1e4 * e64    # rhs TL
    dkit[0:64, 2 * P + 64:3 * P] = e64   # lhsT BR: col p = e_{p-64}
    dkit[0:64, 3 * P + 64:4 * P] = -1e4 * e64
    dkit[64:128] = dkit[0:64]
    zwin_bf = zwin.astype(bf)
    dkit_bf = dkit.astype(bf)

    in_maps = []
    for r in range(NCORES):
        rows = slice(r * RPC, (r + 1) * RPC)
        xl = np.roll(xs, -r * RPC, axis=1)[:, :XTW]            # [64, XTW]
        xt2 = np.concatenate([xl, xl], axis=0).astype(bf)      # [128, XTW]
        xr = X[rows].reshape(NBLK, P, C).transpose(1, 0, 2).reshape(P, NBLK * C)
        gr = G[rows].reshape(NBLK, P, C).transpose(1, 0, 2).reshape(P, NBLK * C)
        in_maps.append({
            "xt2": np.ascontiguousarray(xt2),
            "xrows": np.ascontiguousarray(xr.astype(bf)),
            "g": np.ascontiguousarray(gr.astype(bf)),
            "zwin": zwin_bf,
            "dkit": dkit_bf,
        })
    host = {"cnt": cnt, "nrm": nrm, "logit": logit}
    return in_maps, host


def _combine(results, host):
    se = np.zeros(N, np.float64)
    p_dot = np.zeros(N, np.float64)
    cesum = np.zeros(N, np.float64)
    idx = np.arange(512)
    for r, core_out in enumerate(results):
        st = core_out["stats"].astype(np.float64)
        cs = (core_out["csa"].astype(np.float64)
              + core_out["csb"].astype(np.float64))
        for w in range(NBLK):
            rows = slice((8 * r + w) * P, (8 * r + w) * P + P)
            se[rows] += st[:, w * 4:w * 4 + 4].sum(axis=1)
            p_dot[rows] = st[:, 32 + w]
            cesum[rows] = st[:, 40 + w]
            for t in range(STRIPS):
                gbase = r * RPC + w * P + P + 512 * t
                gi = (gbase + idx) % N
                se[gi] += cs[STRIPS * w + t]
    lse = np.log(se) + SHIFT
    possum = (p_dot - host["nrm"]) / TAU
    npos = host["cnt"] - 1.0
    per_i = np.where(
        npos > 0, lse - possum / np.maximum(npos, 1.0), 0.0
    )
    sc = per_i.sum()
    ce = (np.log(cesum) - host["logit"]).mean()
    return np.float32((1.0 - LMBD) * ce + LMBD * sc)


def run(input, target, trace=False, **build_kw):
    """Run the device kernel; returns (loss_scalar, BassKernelResults)."""
    from concourse.bass_utils import run_bass_kernel_spmd

    nc = _get_nc(**build_kw)
    in_maps, host = _prep(input, target)
    res = run_bass_kernel_spmd(
        nc, in_maps, core_ids=list(range(NCORES)), trace=trace
    )
    loss = _combine(res.results, host)
    return loss, res


def kernel(input, target):
    loss, _ = run(input, target, trace=False)
    return loss


# revision 15
# speedup vs baseline: 1.0193x; 1.0193x over previous
"""Fused CE + supervised-contrastive loss on 8 Trainium2 NeuronCores.

Math (reference semantics):
  ce   = -mean_i log_softmax(input)[i, y_i]
  sim  = (X @ X.T) / tau, diag excluded
  lse_i = logsumexp_{k!=i} sim[i,k]
  possum_i = (x_i . S_{y_i} - ||x_i||^2)/tau,  S_c = sum_{y_k=c} x_k
  per_i = lse_i - possum_i/n_pos_i (0 if n_pos_i == 0)
  loss = (1-lmbd)*ce + lmbd * sum_i per_i

Strategy (v2, symmetric-pair + tile-parallel PE):
  The only O(N^2) work is the exp row-sums se_i = sum_k exp(sim_ik - SHIFT).
  Each core owns 8 contiguous 128-row blocks. Row-block I processes a
  wrap-around window of W=40 column blocks j=0..39:
    - j=0: diagonal block, self-sim killed in PSUM by two 64-contraction
      "eye" accumulate-matmuls (quadrant trick keeps 64x128 tile mode).
    - j=1..24 ("symmetric"): exp'd once; row-sums via ACT/DVE accumulate,
      column-sums (the mirrored pairs' row-sums) via ones-at-s packed
      matmuls: lhsT has its ones in column s, so PSUM row s accumulates the
      strip's colsum while other rows get +0 -- 48 strips pack into one
      [128,512] PSUM tile per contraction half, drained once at the end.
    - j=25..39 ("both-directions"): distances 25..32 are computed from both
      sides (j=d and j=64-d), so no colsum is needed.
  Every matmul is 64-contraction in 64x128 tile mode: T0 instructions use
  SBUF partitions 0-63, T8 partitions 64-127, and the two tiles execute
  CONCURRENTLY (measured 2x; mode switches wedge the device, so the kernel
  never leaves 64x128 mode).  X^T/sqrt(tau) is duplicated into both
  partition halves so any chunk can go to either tile.

  exp work is split ACT (exact spline exp, ~1.2ns/col incl. overhead) vs
  DVE (Schraudolph bf16 fast-exp: clamp+scale, +bias -> int16 bitcast,
  reduce; ~2.9ns/col) to run both engines in parallel.  DVE terms carry a
  calibrated ~0.2% bias -> ~1e-5 on the final loss (tolerance 2e-2).

  Host does O(N*C) prep/post: class sums S -> G=S[y] upload, and the final
  per-row assembly (ln(se), possum, CE combine) in float64.
"""

import numpy as np

N, C = 8192, 64
NCORES = 8
RPC = N // NCORES          # 1024 rows per core
P = 128
NBLK = RPC // P            # 8 row-blocks per core
TAU = 0.5
LMBD = 0.5
SHIFT = 100.0

SYMJ = 28                  # colsum distances 1..SYMJ
WINB = 64 - SYMJ           # 36 window blocks (j = 0..35)
WCOLS = WINB * P           # 4608 window cols
CS_COLS = SYMJ * P         # 3584 colsum cols -> 7 strips of 512
STRIPS = CS_COLS // 512    # 7
XTW = (NBLK - 1) * P + WCOLS  # 7*128 + 4608 = 5504

# window = 3 chunks of 1536; chunk->tile alternates with window parity so
# both tiles carry equal sim+diag work over the 8 windows
C0, C1, C2 = 1536, 1536, 1536
ACT_PRE = 256              # ACT takes c1[0:256], DVE c1[256:1536]

# Schraudolph bf16 fast-exp constants (calibrated)
A16 = 128.0 / float(np.log(2.0))
SIGMA = 3.25
B16P = 16256.0 - SIGMA - SHIFT * A16
CLAMP_LO = SHIFT - 88.0    # 12.0

_CACHE = {}


def _build():
    from contextlib import ExitStack

    import concourse.bass as bass
    import concourse.tile as tile
    from concourse import bacc, mybir

    f32 = mybir.dt.float32
    i16 = mybir.dt.int16
    bf16 = mybir.dt.bfloat16
    AF = mybir.ActivationFunctionType
    ALU = mybir.AluOpType
    AX = mybir.AxisListType

    nc = bacc.Bacc(
        "TRN2",
        target_bir_lowering=False,
        debug=False,
        num_devices=NCORES,
    )

    xt2_d = nc.dram_tensor("xt2", [P, XTW], bf16, kind="ExternalInput")
    xrows_d = nc.dram_tensor("xrows", [P, NBLK * C], bf16, kind="ExternalInput")
    g_d = nc.dram_tensor("g", [P, NBLK * C], bf16, kind="ExternalInput")
    zwin_d = nc.dram_tensor("zwin", [P, 184], bf16, kind="ExternalInput")
    dkit_d = nc.dram_tensor("dkit", [P, 4 * P], bf16, kind="ExternalInput")
    stats_d = nc.dram_tensor("stats", [P, 48], f32, kind="ExternalOutput")
    csa_d = nc.dram_tensor("csa", [P, 512], f32, kind="ExternalOutput")
    csb_d = nc.dram_tensor("csb", [P, 512], f32, kind="ExternalOutput")

    def emit(tc, ctx):
        const = ctx.enter_context(tc.tile_pool(name="const", bufs=1))
        psum = ctx.enter_context(tc.tile_pool(name="psum", bufs=1, space="PSUM"))
        epool = ctx.enter_context(tc.tile_pool(name="epool", bufs=2))
        m1p = ctx.enter_context(tc.tile_pool(name="m1p", bufs=2))
        pmp = ctx.enter_context(tc.tile_pool(name="pmp", bufs=2))
        stats = ctx.enter_context(tc.tile_pool(name="stats", bufs=1))

        # ---- input DMAs, ordered so window 0's operands land first ----
        xt2_sb = const.tile([P, XTW], bf16)
        nc.sync.dma_start(xt2_sb[:, 0:640], xt2_d.ap()[:, 0:640])
        nc.sync.dma_start(xt2_sb[:, 640:1664], xt2_d.ap()[:, 640:1664])
        dkit_sb = const.tile([P, 4 * P], bf16)
        nc.sync.dma_start(dkit_sb[:], dkit_d.ap())
        nc.sync.dma_start(xt2_sb[:, 1664:3712], xt2_d.ap()[:, 1664:3712])
        zwin_sb = const.tile([P, 184], bf16)
        nc.sync.dma_start(zwin_sb[:], zwin_d.ap())
        nc.sync.dma_start(xt2_sb[:, 3712:XTW], xt2_d.ap()[:, 3712:XTW])
        xrows_sb = const.tile([P, NBLK * C], bf16)
        nc.sync.dma_start(xrows_sb[:], xrows_d.ap())
        g_sb = const.tile([P, NBLK * C], bf16)
        nc.sync.dma_start(g_sb[:], g_d.ap())

        # ---- persistent tiles ----
        nshift = stats.tile([P, 1], f32)
        nc.vector.memset(nshift[:], -SHIFT)
        st = stats.tile([P, 48], f32)
        # prefetch the exp spline tables during the DMA head
        warm = stats.tile([P, 1], f32)
        nc.scalar.activation(warm[:], nshift[:], AF.Exp)

        pT0 = psum.tile([P, C0], f32, name="pT0")
        pT8 = psum.tile([P, C1], f32, name="pT8")
        csA = psum.tile([P, 512], f32, name="csA")
        csB = psum.tile([P, 512], f32, name="csB")

        dk = dkit_sb

        def colsum_strips(w, E):
            # strips for window w read E regions produced ~a window ago, so
            # emitting them inside window w+1 keeps the in-order PE queue
            # from stalling on the freshest exp region
            for t in range(STRIPS):
                s = STRIPS * w + t
                a = P + 512 * t
                nc.tensor.matmul(
                    csA[:], lhsT=zwin_sb[0:64, 55 - s:55 - s + P],
                    rhs=E[0:64, a:a + 512],
                    start=(s == 0), stop=(s == STRIPS * NBLK - 1),
                    skip_group_check=True,
                )
                nc.tensor.matmul(
                    csB[:], lhsT=zwin_sb[64:128, 55 - s:55 - s + P],
                    rhs=E[64:128, a:a + 512],
                    start=(s == 0), stop=(s == STRIPS * NBLK - 1),
                    skip_group_check=True,
                )

        prevE = None
        for w in range(NBLK):
            base = w * P
            E = epool.tile([P, WCOLS], bf16, tag="E", name="E")
            Ei = E[:].bitcast(i16)
            lo, hi = (0, 64) if w % 2 == 0 else (64, 128)
            pA, pB = (pT0, pT8) if w % 2 == 0 else (pT8, pT0)
            lhsA = xt2_sb[lo:hi, base:base + P]
            lhsB = xt2_sb[128 - hi:128 - lo, base:base + P]

            # c0 (tile A): window cols [0:1536)
            for k in range(3):
                a = k * 512
                nc.tensor.matmul(
                    pA[:, a:a + 512], lhsT=lhsA,
                    rhs=xt2_sb[lo:hi, base + a:base + a + 512],
                    start=True, stop=True,
                )
            # diagonal kill on cols [0:128), same contraction half as c0
            nc.tensor.matmul(
                pA[:, 0:P], lhsT=dkit_sb[lo:hi, 0:P],
                rhs=dkit_sb[lo:hi, P:2 * P],
                start=False, stop=True, skip_group_check=True,
            )
            nc.tensor.matmul(
                pA[:, 0:P], lhsT=dkit_sb[lo:hi, 2 * P:3 * P],
                rhs=dkit_sb[lo:hi, 3 * P:4 * P],
                start=False, stop=True, skip_group_check=True,
            )
            # c1 (tile B): window cols [1536:3072)
            for k in range(3):
                a = k * 512
                nc.tensor.matmul(
                    pB[:, a:a + 512], lhsT=lhsB,
                    rhs=xt2_sb[128 - hi:128 - lo, base + C0 + a:base + C0 + a + 512],
                    start=True, stop=True,
                )
            # ACT exp c0 -> E[0:1536)
            nc.scalar.activation(
                E[:, 0:C0], pA[:], AF.Exp, bias=nshift[:],
                accum_out=st[:, w * 4:w * 4 + 1],
            )
            # ACT exp c1 prefix [1536:1792)
            nc.scalar.activation(
                E[:, C0:C0 + ACT_PRE], pB[:, 0:ACT_PRE], AF.Exp,
                bias=nshift[:], accum_out=st[:, w * 4 + 1:w * 4 + 2],
            )
            # DVE fast-exp c1 suffix [1792:3072)
            m1a = m1p.tile([P, C1 - ACT_PRE], f32, tag="m1a", name="m1a")
            nc.vector.tensor_scalar(
                out=m1a[:], in0=pB[:, ACT_PRE:C1],
                scalar1=CLAMP_LO, scalar2=A16, op0=ALU.max, op1=ALU.mult,
            )
            nc.vector.tensor_scalar(
                out=Ei[:, C0 + ACT_PRE:C0 + C1], in0=m1a[:],
                scalar1=B16P, scalar2=None, op0=ALU.add,
            )
            nc.vector.reduce_sum(
                st[:, w * 4 + 2:w * 4 + 3], E[:, C0 + ACT_PRE:C0 + C1],
                axis=AX.X,
            )
            # c2 (tile A): window cols [3072:4608)
            for k in range(3):
                a = k * 512
                nc.tensor.matmul(
                    pA[:, a:a + 512], lhsT=lhsA,
                    rhs=xt2_sb[lo:hi, base + C0 + C1 + a:base + C0 + C1 + a + 512],
                    start=True, stop=True,
                )
            nc.scalar.activation(
                E[:, C0 + C1:WCOLS], pA[:], AF.Exp,
                bias=nshift[:], accum_out=st[:, w * 4 + 3:w * 4 + 4],
            )
            # previous window's colsum strips fill the PE while ACT drains c2
            if prevE is not None:
                colsum_strips(w - 1, prevE)
            prevE = E
            # possum dot: x_i . G_i for this block
            pm = pmp.tile([P, C], f32, tag="pm", name="pm")
            nc.gpsimd.tensor_mul(
                pm[:], xrows_sb[:, w * C:(w + 1) * C],
                g_sb[:, w * C:(w + 1) * C],
            )
            nc.vector.reduce_sum(st[:, 32 + w:33 + w], pm[:], axis=AX.X)

            if w == 1:
                # CE denominators, emitted early so they hide under the PE
                cescr = epool.tile([P, NBLK * C], f32, tag="cescr",
                                   name="cescr")
                nc.scalar.activation(cescr[:], xrows_sb[:], AF.Exp)
                for v in range(NBLK):
                    nc.vector.reduce_sum(
                        st[:, 40 + v:41 + v], cescr[:, v * C:(v + 1) * C],
                        axis=AX.X,
                    )

        colsum_strips(NBLK - 1, prevE)

        nc.sync.dma_start(stats_d.ap(), st[:])
        csa_sb = stats.tile([P, 512], f32)
        nc.vector.tensor_copy(csa_sb[:], csA[:])
        nc.sync.dma_start(csa_d.ap(), csa_sb[:])
        csb_sb = stats.tile([P, 512], f32)
        nc.vector.tensor_copy(csb_sb[:], csB[:])
        nc.sync.dma_start(csb_d.ap(), csb_sb[:])

    with tile.TileContext(nc) as tc, ExitStack() as ctx:
        emit(tc, ctx)

    nc.compile()
    return nc


def _get_nc(**kw):
    key = repr(sorted(kw.items()))
    if key not in _CACHE:
        _CACHE[key] = _build(**kw)
    return _CACHE[key]


def _prep(X, y):
    import ml_dtypes

    bf = ml_dtypes.bfloat16
    X = np.ascontiguousarray(np.asarray(X, dtype=np.float32))
    y = np.asarray(y).astype(np.int64).ravel()
    assert X.shape == (N, C) and y.shape == (N,)

    xs = (X.T / np.float32(np.sqrt(TAU))).astype(np.float32)  # [C, N]

    # class sums and per-row gathers (host, O(N*C))
    S = np.zeros((C, C), np.float64)
    np.add.at(S, y, X.astype(np.float64))
    G = S[y].astype(np.float32)                                # [N, C]
    cnt = np.bincount(y, minlength=C)[y].astype(np.float64)    # incl self
    nrm = (X.astype(np.float64) ** 2).sum(1)
    logit = X[np.arange(N), y].astype(np.float64)

    # ones-at-s window for colsum packing: col 55 is all-ones
    zwin = np.zeros((P, 184), np.float32)
    zwin[:, 55] = 1.0

    # diag-kill quadrants (64-contraction), duplicated into both halves so
    # either tile's contraction range can apply them
    dkit = np.zeros((P, 4 * P), np.float32)
    e64 = np.eye(64, dtype=np.float32)
    dkit[0:64, 0:64] = e64               # lhsT TL: col p = e_p (p<64)
    dkit[0:64, P:P + 64] = -1e4 * e64    # rhs TL
    dkit[0:64, 2 * P + 64:3 * P] = e64   # lhsT BR: col p = e_{p-64}
    dkit[0:64, 3 * P + 64:4 * P] = -1e4 * e64
    dkit[64:128] = dkit[0:64]
    zwin_bf = zwin.astype(bf)
    dkit_bf = dkit.astype(bf)

    in_maps = []
    for r in range(NCORES):
        rows = slice(r * RPC, (r + 1) * RPC)
        xl = np.roll(xs, -r * RPC, axis=1)[:, :XTW]            # [64, XTW]
        xt2 = np.concatenate([xl, xl], axis=0).astype(bf)      # [128, XTW]
        xr = X[rows].reshape(NBLK, P, C).transpose(1, 0, 2).reshape(P, NBLK * C)
        gr = G[rows].reshape(NBLK, P, C).transpose(1, 0, 2).reshape(P, NBLK * C)
        in_maps.append({
            "xt2": np.ascontiguousarray(xt2),
            "xrows": np.ascontiguousarray(xr.astype(bf)),
            "g": np.ascontiguousarray(gr.astype(bf)),
            "zwin": zwin_bf,
            "dkit": dkit_bf,
        })
    host = {"cnt": cnt, "nrm": nrm, "logit": logit}
    return in_maps, host


def _combine(results, host):
    se = np.zeros(N, np.float64)
    p_dot = np.zeros(N, np.float64)
    cesum = np.zeros(N, np.float64)
    idx = np.arange(512)
    for r, core_out in enumerate(results):
        st = core_out["stats"].astype(np.float64)
        cs = (core_out["csa"].astype(np.float64)
              + core_out["csb"].astype(np.float64))
        for w in range(NBLK):
            rows = slice((8 * r + w) * P, (8 * r + w) * P + P)
            se[rows] += st[:, w * 4:w * 4 + 4].sum(axis=1)
            p_dot[rows] = st[:, 32 + w]
            cesum[rows] = st[:, 40 + w]
            for t in range(STRIPS):
                gbase = r * RPC + w * P + P + 512 * t
                gi = (gbase + idx) % N
                se[gi] += cs[STRIPS * w + t]
    lse = np.log(se) + SHIFT
    possum = (p_dot - host["nrm"]) / TAU
    npos = host["cnt"] - 1.0
    per_i = np.where(
        npos > 0, lse - possum / np.maximum(npos, 1.0), 0.0
    )
    sc = per_i.sum()
    ce = (np.log(cesum) - host["logit"]).mean()
    return np.float32((1.0 - LMBD) * ce + LMBD * sc)


def run(input, target, trace=False, **build_kw):
    """Run the device kernel; returns (loss_scalar, BassKernelResults)."""
    from concourse.bass_utils import run_bass_kernel_spmd

    nc = _get_nc(**build_kw)
    in_maps, host = _prep(input, target)
    res = run_bass_kernel_spmd(
        nc, in_maps, core_ids=list(range(NCORES)), trace=trace
    )
    loss = _combine(res.results, host)
    return loss, res


def kernel(input, target):
    loss, _ = run(input, target, trace=False)
    return loss


# revision 16
# speedup vs baseline: 1.0503x; 1.0305x over previous
"""Fused CE + supervised-contrastive loss on 8 Trainium2 NeuronCores.

Math (reference semantics):
  ce   = -mean_i log_softmax(input)[i, y_i]
  sim  = (X @ X.T) / tau, diag excluded
  lse_i = logsumexp_{k!=i} sim[i,k]
  possum_i = (x_i . S_{y_i} - ||x_i||^2)/tau,  S_c = sum_{y_k=c} x_k
  per_i = lse_i - possum_i/n_pos_i (0 if n_pos_i == 0)
  loss = (1-lmbd)*ce + lmbd * sum_i per_i

Strategy (v2, symmetric-pair + tile-parallel PE):
  The only O(N^2) work is the exp row-sums se_i = sum_k exp(sim_ik - SHIFT).
  Each core owns 8 contiguous 128-row blocks. Row-block I processes a
  wrap-around window of W=40 column blocks j=0..39:
    - j=0: diagonal block, self-sim killed in PSUM by two 64-contraction
      "eye" accumulate-matmuls (quadrant trick keeps 64x128 tile mode).
    - j=1..24 ("symmetric"): exp'd once; row-sums via ACT/DVE accumulate,
      column-sums (the mirrored pairs' row-sums) via ones-at-s packed
      matmuls: lhsT has its ones in column s, so PSUM row s accumulates the
      strip's colsum while other rows get +0 -- 48 strips pack into one
      [128,512] PSUM tile per contraction half, drained once at the end.
    - j=25..39 ("both-directions"): distances 25..32 are computed from both
      sides (j=d and j=64-d), so no colsum is needed.
  Every matmul is 64-contraction in 64x128 tile mode: T0 instructions use
  SBUF partitions 0-63, T8 partitions 64-127, and the two tiles execute
  CONCURRENTLY (measured 2x; mode switches wedge the device, so the kernel
  never leaves 64x128 mode).  X^T/sqrt(tau) is duplicated into both
  partition halves so any chunk can go to either tile.

  exp work is split ACT (exact spline exp, ~1.2ns/col incl. overhead) vs
  DVE (Schraudolph bf16 fast-exp: clamp+scale, +bias -> int16 bitcast,
  reduce; ~2.9ns/col) to run both engines in parallel.  DVE terms carry a
  calibrated ~0.2% bias -> ~1e-5 on the final loss (tolerance 2e-2).

  Host does O(N*C) prep/post: class sums S -> G=S[y] upload, and the final
  per-row assembly (ln(se), possum, CE combine) in float64.
"""

import numpy as np

N, C = 8192, 64
NCORES = 8
RPC = N // NCORES          # 1024 rows per core
P = 128
NBLK = RPC // P            # 8 row-blocks per core
TAU = 0.5
LMBD = 0.5
SHIFT = 100.0

SYMJ = 28                  # colsum distances 1..SYMJ
WINB = 64 - SYMJ           # 36 window blocks (j = 0..35)
WCOLS = WINB * P           # 4608 window cols
CS_COLS = SYMJ * P         # 3584 colsum cols -> 7 strips of 512
STRIPS = CS_COLS // 512    # 7
XTW = (NBLK - 1) * P + WCOLS  # 7*128 + 4608 = 5504

# window = 3 chunks of 1536; chunk->tile alternates with window parity so
# both tiles carry equal sim+diag work over the 8 windows
C0, C1, C2 = 1536, 1536, 1536
ACT_PRE = 256              # ACT takes c1[0:256], DVE c1[256:1536]

# Schraudolph bf16 fast-exp constants (calibrated)
A16 = 128.0 / float(np.log(2.0))
SIGMA = 3.25
B16P = 16256.0 - SIGMA - SHIFT * A16
CLAMP_LO = SHIFT - 88.0    # 12.0

_CACHE = {}


def _build():
    from contextlib import ExitStack

    import concourse.bass as bass
    import concourse.tile as tile
    from concourse import bacc, mybir

    f32 = mybir.dt.float32
    i16 = mybir.dt.int16
    bf16 = mybir.dt.bfloat16
    AF = mybir.ActivationFunctionType
    ALU = mybir.AluOpType
    AX = mybir.AxisListType

    nc = bacc.Bacc(
        "TRN2",
        target_bir_lowering=False,
        debug=False,
        num_devices=NCORES,
    )

    xt2_d = nc.dram_tensor("xt2", [P, XTW], bf16, kind="ExternalInput")
    xrows_d = nc.dram_tensor("xrows", [P, NBLK * C], bf16, kind="ExternalInput")
    g_d = nc.dram_tensor("g", [P, NBLK * C], bf16, kind="ExternalInput")
    zwin_d = nc.dram_tensor("zwin", [P, 184], bf16, kind="ExternalInput")
    dkit_d = nc.dram_tensor("dkit", [P, 4 * P], bf16, kind="ExternalInput")
    stats_d = nc.dram_tensor("stats", [P, 48], f32, kind="ExternalOutput")
    csa_d = nc.dram_tensor("csa", [P, 512], f32, kind="ExternalOutput")
    csb_d = nc.dram_tensor("csb", [P, 512], f32, kind="ExternalOutput")

    def emit(tc, ctx):
        const = ctx.enter_context(tc.tile_pool(name="const", bufs=1))
        psum = ctx.enter_context(tc.tile_pool(name="psum", bufs=1, space="PSUM"))
        epool = ctx.enter_context(tc.tile_pool(name="epool", bufs=2))
        m1p = ctx.enter_context(tc.tile_pool(name="m1p", bufs=2))
        pmp = ctx.enter_context(tc.tile_pool(name="pmp", bufs=2))
        stats = ctx.enter_context(tc.tile_pool(name="stats", bufs=1))

        # ---- input DMAs, ordered so window 0's operands land first ----
        xt2_sb = const.tile([P, XTW], bf16)
        nc.sync.dma_start(xt2_sb[:, 0:1664], xt2_d.ap()[:, 0:1664])
        dkit_sb = const.tile([P, 4 * P], bf16)
        nc.sync.dma_start(dkit_sb[:], dkit_d.ap())
        nc.sync.dma_start(xt2_sb[:, 1664:3712], xt2_d.ap()[:, 1664:3712])
        zwin_sb = const.tile([P, 184], bf16)
        nc.sync.dma_start(zwin_sb[:], zwin_d.ap())
        nc.sync.dma_start(xt2_sb[:, 3712:XTW], xt2_d.ap()[:, 3712:XTW])
        xrows_sb = const.tile([P, NBLK * C], bf16)
        nc.sync.dma_start(xrows_sb[:], xrows_d.ap())
        g_sb = const.tile([P, NBLK * C], bf16)
        nc.sync.dma_start(g_sb[:], g_d.ap())

        # ---- persistent tiles ----
        nshift = stats.tile([P, 1], f32)
        nc.vector.memset(nshift[:], -SHIFT)
        st = stats.tile([P, 48], f32)
        # prefetch the exp spline tables during the DMA head
        warm = stats.tile([P, 1], f32)
        nc.scalar.activation(warm[:], nshift[:], AF.Exp)

        pT0 = psum.tile([P, C0], f32, name="pT0")
        pT8 = psum.tile([P, C1], f32, name="pT8")
        csA = psum.tile([P, 512], f32, name="csA")
        csB = psum.tile([P, 512], f32, name="csB")

        dk = dkit_sb

        def colsum_strips(w, E):
            # strips for window w read E regions produced ~a window ago, so
            # emitting them inside window w+1 keeps the in-order PE queue
            # from stalling on the freshest exp region
            for t in range(STRIPS):
                s = STRIPS * w + t
                a = P + 512 * t
                nc.tensor.matmul(
                    csA[:], lhsT=zwin_sb[0:64, 55 - s:55 - s + P],
                    rhs=E[0:64, a:a + 512],
                    start=(s == 0), stop=(s == STRIPS * NBLK - 1),
                    skip_group_check=True,
                )
                nc.tensor.matmul(
                    csB[:], lhsT=zwin_sb[64:128, 55 - s:55 - s + P],
                    rhs=E[64:128, a:a + 512],
                    start=(s == 0), stop=(s == STRIPS * NBLK - 1),
                    skip_group_check=True,
                )

        prevE = None
        for w in range(NBLK):
            base = w * P
            E = epool.tile([P, WCOLS], bf16, tag="E", name="E")
            Ei = E[:].bitcast(i16)
            lo, hi = (0, 64) if w % 2 == 0 else (64, 128)
            pA, pB = (pT0, pT8) if w % 2 == 0 else (pT8, pT0)
            lhsA = xt2_sb[lo:hi, base:base + P]
            lhsB = xt2_sb[128 - hi:128 - lo, base:base + P]

            # c0 (tile A): window cols [0:1536)
            for k in range(3):
                a = k * 512
                nc.tensor.matmul(
                    pA[:, a:a + 512], lhsT=lhsA,
                    rhs=xt2_sb[lo:hi, base + a:base + a + 512],
                    start=True, stop=True,
                )
            # diagonal kill on cols [0:128), same contraction half as c0
            nc.tensor.matmul(
                pA[:, 0:P], lhsT=dkit_sb[lo:hi, 0:P],
                rhs=dkit_sb[lo:hi, P:2 * P],
                start=False, stop=True, skip_group_check=True,
            )
            nc.tensor.matmul(
                pA[:, 0:P], lhsT=dkit_sb[lo:hi, 2 * P:3 * P],
                rhs=dkit_sb[lo:hi, 3 * P:4 * P],
                start=False, stop=True, skip_group_check=True,
            )
            # c1 (tile B): window cols [1536:3072)
            for k in range(3):
                a = k * 512
                nc.tensor.matmul(
                    pB[:, a:a + 512], lhsT=lhsB,
                    rhs=xt2_sb[128 - hi:128 - lo, base + C0 + a:base + C0 + a + 512],
                    start=True, stop=True,
                )
            # ACT exp c0 -> E[0:1536)
            nc.scalar.activation(
                E[:, 0:C0], pA[:], AF.Exp, bias=nshift[:],
                accum_out=st[:, w * 4:w * 4 + 1],
            )
            # ACT exp c1 prefix [1536:1792)
            nc.scalar.activation(
                E[:, C0:C0 + ACT_PRE], pB[:, 0:ACT_PRE], AF.Exp,
                bias=nshift[:], accum_out=st[:, w * 4 + 1:w * 4 + 2],
            )
            # DVE fast-exp c1 suffix [1792:3072)
            m1a = m1p.tile([P, C1 - ACT_PRE], f32, tag="m1a", name="m1a")
            nc.vector.tensor_scalar(
                out=m1a[:], in0=pB[:, ACT_PRE:C1],
                scalar1=CLAMP_LO, scalar2=A16, op0=ALU.max, op1=ALU.mult,
            )
            nc.vector.tensor_scalar(
                out=Ei[:, C0 + ACT_PRE:C0 + C1], in0=m1a[:],
                scalar1=B16P, scalar2=None, op0=ALU.add,
            )
            nc.vector.reduce_sum(
                st[:, w * 4 + 2:w * 4 + 3], E[:, C0 + ACT_PRE:C0 + C1],
                axis=AX.X,
            )
            # c2 (tile A): window cols [3072:4608)
            for k in range(3):
                a = k * 512
                nc.tensor.matmul(
                    pA[:, a:a + 512], lhsT=lhsA,
                    rhs=xt2_sb[lo:hi, base + C0 + C1 + a:base + C0 + C1 + a + 512],
                    start=True, stop=True,
                )
            nc.scalar.activation(
                E[:, C0 + C1:WCOLS], pA[:], AF.Exp,
                bias=nshift[:], accum_out=st[:, w * 4 + 3:w * 4 + 4],
            )
            # previous window's colsum strips fill the PE while ACT drains c2
            if prevE is not None:
                colsum_strips(w - 1, prevE)
            prevE = E
            # possum dot: x_i . G_i for this block
            pm = pmp.tile([P, C], f32, tag="pm", name="pm")
            nc.gpsimd.tensor_mul(
                pm[:], xrows_sb[:, w * C:(w + 1) * C],
                g_sb[:, w * C:(w + 1) * C],
            )
            nc.vector.reduce_sum(st[:, 32 + w:33 + w], pm[:], axis=AX.X)

            if w == 1:
                # CE denominators, emitted early so they hide under the PE
                cescr = epool.tile([P, NBLK * C], f32, tag="cescr",
                                   name="cescr")
                nc.scalar.activation(cescr[:], xrows_sb[:], AF.Exp)
                for v in range(NBLK):
                    nc.vector.reduce_sum(
                        st[:, 40 + v:41 + v], cescr[:, v * C:(v + 1) * C],
                        axis=AX.X,
                    )

        colsum_strips(NBLK - 1, prevE)

        nc.sync.dma_start(stats_d.ap(), st[:])
        csa_sb = stats.tile([P, 512], f32)
        nc.vector.tensor_copy(csa_sb[:], csA[:])
        nc.sync.dma_start(csa_d.ap(), csa_sb[:])
        csb_sb = stats.tile([P, 512], f32)
        nc.vector.tensor_copy(csb_sb[:], csB[:])
        nc.sync.dma_start(csb_d.ap(), csb_sb[:])

    with tile.TileContext(nc) as tc, ExitStack() as ctx:
        emit(tc, ctx)

    nc.compile()
    return nc


def _get_nc(**kw):
    key = repr(sorted(kw.items()))
    if key not in _CACHE:
        _CACHE[key] = _build(**kw)
    return _CACHE[key]


def _prep(X, y):
    import ml_dtypes

    bf = ml_dtypes.bfloat16
    X = np.ascontiguousarray(np.asarray(X, dtype=np.float32))
    y = np.asarray(y).astype(np.int64).ravel()
    assert X.shape == (N, C) and y.shape == (N,)

    xs = (X.T / np.float32(np.sqrt(TAU))).astype(np.float32)  # [C, N]

    # class sums and per-row gathers (host, O(N*C))
    S = np.zeros((C, C), np.float64)
    np.add.at(S, y, X.astype(np.float64))
    G = S[y].astype(np.float32)                                # [N, C]
    cnt = np.bincount(y, minlength=C)[y].astype(np.float64)    # incl self
    nrm = (X.astype(np.float64) ** 2).sum(1)
    logit = X[np.arange(N), y].astype(np.float64)

    # ones-at-s window for colsum packing: col 55 is all-ones
    zwin = np.zeros((P, 184), np.float32)
    zwin[:, 55] = 1.0

    # diag-kill quadrants (64-contraction), duplicated into both halves so
    # either tile's contraction range can apply them
    dkit = np.zeros((P, 4 * P), np.float32)
    e64 = np.eye(64, dtype=np.float32)
    dkit[0:64, 0:64] = e64               # lhsT TL: col p = e_p (p<64)
    dkit[0:64, P:P + 64] = -1e4 * e64    # rhs TL
    dkit[0:64, 2 * P + 64:3 * P] = e64   # lhsT BR: col p = e_{p-64}
    dkit[0:64, 3 * P + 64:4 * P] = -1e4 * e64
    dkit[64:128] = dkit[0:64]
    zwin_bf = zwin.astype(bf)
    dkit_bf = dkit.astype(bf)

    in_maps = []
    for r in range(NCORES):
        rows = slice(r * RPC, (r + 1) * RPC)
        xl = np.roll(xs, -r * RPC, axis=1)[:, :XTW]            # [64, XTW]
        xt2 = np.concatenate([xl, xl], axis=0).astype(bf)      # [128, XTW]
        xr = X[rows].reshape(NBLK, P, C).transpose(1, 0, 2).reshape(P, NBLK * C)
        gr = G[rows].reshape(NBLK, P, C).transpose(1, 0, 2).reshape(P, NBLK * C)
        in_maps.append({
            "xt2": np.ascontiguousarray(xt2),
            "xrows": np.ascontiguousarray(xr.astype(bf)),
            "g": np.ascontiguousarray(gr.astype(bf)),
            "zwin": zwin_bf,
            "dkit": dkit_bf,
        })
    host = {"cnt": cnt, "nrm": nrm, "logit": logit}
    return in_maps, host


def _combine(results, host):
    se = np.zeros(N, np.float64)
    p_dot = np.zeros(N, np.float64)
    cesum = np.zeros(N, np.float64)
    idx = np.arange(512)
    for r, core_out in enumerate(results):
        st = core_out["stats"].astype(np.float64)
        cs = (core_out["csa"].astype(np.float64)
              + core_out["csb"].astype(np.float64))
        for w in range(NBLK):
            rows = slice((8 * r + w) * P, (8 * r + w) * P + P)
            se[rows] += st[:, w * 4:w * 4 + 4].sum(axis=1)
            p_dot[rows] = st[:, 32 + w]
            cesum[rows] = st[:, 40 + w]
            for t in range(STRIPS):
                gbase = r * RPC + w * P + P + 512 * t
                gi = (gbase + idx) % N
                se[gi] += cs[STRIPS * w + t]
    lse = np.log(se) + SHIFT
    possum = (p_dot - host["nrm"]) / TAU
    npos = host["cnt"] - 1.0
    per_i = np.where(
        npos > 0, lse - possum / np.maximum(npos, 1.0), 0.0
    )
    sc = per_i.sum()
    ce = (np.log(cesum) - host["logit"]).mean()
    return np.float32((1.0 - LMBD) * ce + LMBD * sc)


def run(input, target, trace=False, **build_kw):
    """Run the device kernel; returns (loss_scalar, BassKernelResults)."""
    from concourse.bass_utils import run_bass_kernel_spmd

    nc = _get_nc(**build_kw)
    in_maps, host = _prep(input, target)
    res = run_bass_kernel_spmd(
        nc, in_maps, core_ids=list(range(NCORES)), trace=trace
    )
    loss = _combine(res.results, host)
    return loss, res


def kernel(input, target):
    loss, _ = run(input, target, trace=False)
    return loss
